# revision 4
# baseline (speedup 1.0000x reference)
"""Trainium2 Bass kernel for a GPT-style transformer block.

Reference computation (per batch element):
    h  = LN1(x);  qkv = h @ qkv_w + qkv_b
    att = causal_softmax(q @ k.T / sqrt(64));  o = att @ v
    x  = x + o @ out_w + out_b
    h  = LN2(x);  u = relu(h @ fc_w + fc_b)
    y  = x + u @ proj_w + proj_b

Shapes: x [16, 1024, 256], 4 heads x 64, MLP hidden 1024.

Strategy: pure data-parallel over batch, 2 batch elements per core on 8
cores, no collectives.  Within a core:
  - All PE operands are bf16 (fp32 PSUM accumulation).  LN gamma/beta are
    folded into the following matmul weights on the host.
  - Scores are computed transposed (scoresT[k, q]) with the two heads of a
    pair running concurrently as 64-row PE tiles.  The causal mask is a DVE
    add of a strict-lower -1e9 tile onto the diagonal PSUM block before the
    exp (ACT) evacuates psum -> expT (bf16, ragged trapezoid layout).
  - att@V runs j-outer with the v chunks as 64-column stationaries: the two
    heads of a pair occupy the two column-tile slots of the PE and run
    concurrently, accumulating oT[d, q] over j in PSUM.  Softmax
    denominators accumulate in a second PSUM tile from an all-ones [128,64]
    stationary, which replicates den across all 64 partitions so the
    reciprocal and the normalize multiply are single partition-aligned DVE
    ops.  The normalized output lands directly feature-major (oT) for
    out_proj -- no transpose.
  - The two batch elements are software-pipelined: element 1's scores/exp
    chase element 0's exp stream on ACT, while element 0's attV + MLP fill
    the PE during element 1's exp window.  No filler matmuls.
  - PSUM: tag "big" [128,1024]x2 (qk + scores) and tag "att" [128,1024]x2
    (v, attV accumulators, out_proj, fc, proj) = exactly 8 banks.
    Accumulation regions coincide exactly with bank boundaries.
"""

import sys

sys.path.insert(0, "/opt/trn_rl_repo")

import numpy as np

import concourse.bass as bass
import concourse.bacc as bacc
import concourse.tile as tile
from concourse import mybir

# Restrict the activation-table chooser to the one set that contains every
# function this kernel uses (exp, ln, copy/identity, relu) so the ACT
# engine never thrashes table loads.
if not getattr(bacc.get_activation_tables, "_bass_kernel_patched", False):
    _orig_get_act_tables = bacc.get_activation_tables

    def _one_set_tables(module_arch):
        tabs = _orig_get_act_tables(module_arch)
        return {name: (fns if name == "natural_log_exp_and_others" else set())
                for name, fns in tabs.items()}

    _one_set_tables._bass_kernel_patched = True
    bacc.get_activation_tables = _one_set_tables

F32 = mybir.dt.float32
F32R = mybir.dt.float32r
BF16 = mybir.dt.bfloat16
AF = mybir.ActivationFunctionType
ALU = mybir.AluOpType

import os as _os

MM_DTYPE = _os.environ.get("BASS_MM_DTYPE", "bf16")
WARMUP_MM = int(_os.environ.get("BASS_WARMUP_MM", "14"))


def _mmdt():
    return F32R if MM_DTYPE == "f32r" else BF16

NCORES = 8
B = 16
BPC = B // NCORES  # 2 batch elements per core
S = 1024
E = 256
H = 4
D = 64
FF = 1024
ST = S // 128  # 8 seq tiles
ET = E // 128  # 2 feature tiles
FT = FF // 128  # 8 mlp-hidden tiles
EPS = 1e-5

# ragged offsets for the causal expT store: tile j holds q in [128j, S)
EOFF = [0]
for _j in range(ST):
    EOFF.append(EOFF[-1] + (S - 128 * _j))
ETOT = EOFF[ST]  # 4608


def _r(ap):
    """View an fp32 AP as float32r (no-op for bf16/f32r tiles)."""
    if ap.dtype in (F32R, BF16):
        return ap
    return ap.bitcast(F32R)


def build_bass(reps=1):
    MMDT = _mmdt()
    nc = bacc.Bacc(None, target_bir_lowering=False, debug=False)

    # ---- DRAM I/O ----
    x_in = nc.dram_tensor("x", [BPC, S, E], F32, kind="ExternalInput")
    qk_w = nc.dram_tensor("qk_w", [E, 512], MMDT, kind="ExternalInput")
    qk_bc = nc.dram_tensor("qk_bc", [128, 4], F32, kind="ExternalInput")
    wv = nc.dram_tensor("wv", [E, E], MMDT, kind="ExternalInput")
    bv_row = nc.dram_tensor("bv_row", [1, E], MMDT, kind="ExternalInput")
    out_w = nc.dram_tensor("out_w", [E, E], MMDT, kind="ExternalInput")
    outb_row = nc.dram_tensor("outb_row", [1, E], MMDT, kind="ExternalInput")
    fc_w = nc.dram_tensor("fc_w", [E, FF], MMDT, kind="ExternalInput")
    fc_bt = nc.dram_tensor("fc_bt", [128, FT], F32, kind="ExternalInput")
    proj_w = nc.dram_tensor("proj_w", [FF, E], MMDT, kind="ExternalInput")
    projb_row = nc.dram_tensor("projb_row", [1, E], MMDT, kind="ExternalInput")
    y_out = nc.dram_tensor("y", [BPC, S, E], F32, kind="ExternalOutput")

    with tile.TileContext(nc) as tc:
        wp = tc.alloc_tile_pool(name="weights", bufs=1)
        sp = tc.alloc_tile_pool(name="small", bufs=2)
        bp = tc.alloc_tile_pool(name="big", bufs=2)
        ep = tc.alloc_tile_pool(name="expt", bufs=2)
        psb = tc.alloc_tile_pool(name="psbig", bufs=2, space="PSUM")
        psa = tc.alloc_tile_pool(name="psatt", bufs=2, space="PSUM")

        # ---- constants first: gpsimd builds the mask before anything else ----
        eps_col = wp.tile([128, 1], F32)
        nc.vector.memset(eps_col, EPS)
        ones_row = wp.tile([1, S], MMDT)
        nc.vector.memset(ones_row, 1.0)
        ones_all = wp.tile([128, D], MMDT)
        nc.vector.memset(ones_all, 1.0)
        # additive causal mask for the scoresT diagonal block: -1e9 where
        # q < k (col < partition), 0 elsewhere
        masklow = wp.tile([128, 128], F32)
        nc.gpsimd.memset(masklow, 0.0)
        nc.gpsimd.affine_select(
            out=masklow, in_=masklow, compare_op=ALU.is_ge, fill=-1e9,
            base=0, channel_multiplier=-1, pattern=[[1, 128]])

        # ---- PE warmup: keep the HAM clock gate open during input DMA ----
        warm = wp.tile([128, 512], MMDT)
        nc.vector.memset(warm, 0.25)
        if WARMUP_MM:
            for _ in range(WARMUP_MM):
                wps = psa.tile([128, S], F32, tag="att")
                nc.tensor.matmul(wps[:, 0:512], _r(warm[:, 0:128]), _r(warm),
                                 start=True, stop=True)

        # ---- weights needed early (attention input projections) ----
        qk_w_sb = wp.tile([128, ET, 512], MMDT)
        nc.gpsimd.dma_start(out=qk_w_sb, in_=qk_w[:, :].rearrange("(t p) c -> p t c", p=128))
        qk_bc_sb = wp.tile([128, 4], F32)
        nc.gpsimd.dma_start(out=qk_bc_sb, in_=qk_bc[:, :])
        wv_sb = wp.tile([128, ET, E], MMDT)
        nc.gpsimd.dma_start(out=wv_sb, in_=wv[:, :].rearrange("(t p) c -> p t c", p=128))
        bv_sb = wp.tile([1, E], MMDT)
        nc.gpsimd.dma_start(out=bv_sb, in_=bv_row[:, :])

        # late weights: tiles allocated now, DMAs emitted after p1(0)
        out_w_sb = wp.tile([128, ET, E], MMDT)
        outb_sb = wp.tile([1, E], MMDT)
        fc_w_sb = wp.tile([128, ET, FF], MMDT)
        fc_bt_sb = wp.tile([128, FT], F32)
        proj_w_sb = wp.tile([128, FT, E], MMDT)
        projb_sb = wp.tile([1, E], MMDT)

        def emit_late_weights():
            nc.gpsimd.dma_start(out=out_w_sb, in_=out_w[:, :].rearrange("(t p) c -> p t c", p=128))
            nc.gpsimd.dma_start(out=outb_sb, in_=outb_row[:, :])
            nc.gpsimd.dma_start(out=fc_w_sb, in_=fc_w[:, :].rearrange("(t p) c -> p t c", p=128))
            nc.gpsimd.dma_start(out=fc_bt_sb, in_=fc_bt[:, :])
            nc.gpsimd.dma_start(out=proj_w_sb, in_=proj_w[:, :].rearrange("(t p) c -> p t c", p=128))
            nc.gpsimd.dma_start(out=projb_sb, in_=projb_row[:, :])

        def emit_ln_half(src, dst_fn, half, rstd, nmr):
            """LayerNorm of seq tiles [4*half, 4*half+4): stats on DVE, rstd
            on ACT, apply on GpSimd -> bf16 dst."""
            t0 = 4 * half
            stats = sp.tile([128, 4, 6], F32, tag="bnstats")
            mv = sp.tile([128, 4, 2], F32, tag="bnaggr")
            for i in range(4):
                nc.vector.bn_stats(out=stats[:, i, :], in_=src[:, t0 + i, :])
                nc.vector.bn_aggr(out=mv[:, i, :], in_=stats[:, i, :])
            sl = slice(t0, t0 + 4)
            # rstd = exp(-0.5 * ln(var + eps))
            nc.scalar.activation(rstd[:, sl], mv[:, :, 1], AF.Ln, bias=eps_col)
            nc.scalar.activation(rstd[:, sl], rstd[:, sl], AF.Exp, scale=-0.5)
            nc.vector.tensor_mul(nmr[:, sl], mv[:, :, 0], rstd[:, sl])
            nc.vector.tensor_scalar_mul(nmr[:, sl], nmr[:, sl], -1.0)
            for i in range(4):
                t = t0 + i
                nc.gpsimd.tensor_scalar(
                    out=dst_fn(t), in0=src[:, t, :],
                    scalar1=rstd[:, t:t + 1], scalar2=nmr[:, t:t + 1],
                    op0=ALU.mult, op1=ALU.add)

        def emit_transpose_half(h_sb, hT, half):
            """XBAR transpose seq tiles [4h, 4h+4) of h_sb [128, ST, E] into
            hT [128, ST, ET, 128] (layout [e_lo, t, et, q])."""
            t0 = 4 * half
            nc.sync.dma_start_transpose(
                out=hT[:, t0:t0 + 4, :, :], in_=h_sb[:, t0:t0 + 4, :])

        def emit_ln(src, name):
            """Full LN -> (h bf16, hT transposed) using ring tags h/hT."""
            h = bp.tile([128, ST, E], MMDT, tag="h", name=f"h{name}")
            hT = bp.tile([128, ST, ET, 128], MMDT, tag="hT", name=f"hT{name}")
            rstd = sp.tile([128, ST], F32, tag="rstd")
            nmr = sp.tile([128, ST], F32, tag="nmr")
            for half in (0, 1):
                emit_ln_half(src, lambda t: h[:, t, :], half, rstd, nmr)
                emit_transpose_half(h, hT, half)
            return hT

        def emit_qk(b, hT, qkT):
            # q/k feature-major: qkT[:, m, :]; m=0: q heads 0-1, m=1: q heads
            # 2-3, m=2: k heads 0-1, m=3: k heads 2-3 (rows = 2x64 head dims)
            for m in range(4):
                ps = psb.tile([128, S], F32, tag="big", name=f"qk{b}m{m}")
                for c in range(2):
                    sl = slice(512 * c, 512 * (c + 1))
                    nc.tensor.matmul(ps[:, sl], _r(qk_w_sb[:, 0, 128 * m:128 * (m + 1)]),
                                     _r(hT[:, 4 * c:4 * c + 4, 0, :]),
                                     start=True, stop=False)
                    nc.tensor.matmul(ps[:, sl], _r(qk_w_sb[:, 1, 128 * m:128 * (m + 1)]),
                                     _r(hT[:, 4 * c:4 * c + 4, 1, :]),
                                     start=False, stop=True)
                    nc.vector.tensor_scalar(
                        out=qkT[:, m, sl], in0=ps[:, sl],
                        scalar1=qk_bc_sb[:, m:m + 1],
                        scalar2=None, op0=ALU.add)

        def emit_v(b, hT, vv, trng):
            # v seq-major: vv[:, t, 64h:64h+64] per head
            for t in trng:
                ps = psa.tile([128, S], F32, tag="att", name=f"v{b}t{t}")
                nc.tensor.matmul(ps[:, 0:E], _r(hT[:, t, 0, :]),
                                 _r(wv_sb[:, 0, :]), start=True, stop=False)
                nc.tensor.matmul(ps[:, 0:E], _r(hT[:, t, 1, :]),
                                 _r(wv_sb[:, 1, :]), start=False, stop=False)
                nc.tensor.matmul(ps[:, 0:E], _r(ones_row[0:1, 128 * t:128 * (t + 1)]),
                                 _r(bv_sb[0:1, :]), start=False, stop=True)
                nc.vector.tensor_copy(vv[:, t, :], ps[:, 0:E])

        def emit_scores_j(b, j, qkT, expTs):
            """Scores + mask + exp for k-tile j (both pairs)."""
            w0 = 128 * j
            for pair in range(2):
                pss = {}
                for hh in range(2):
                    h = 2 * pair + hh
                    qsl = slice(64 * hh, 64 * hh + 64)
                    ps = psb.tile([128, S], F32, tag="big", name=f"sc{b}h{h}j{j}")
                    pss[h] = ps
                    for c in range(w0 // 512, 2):
                        a = max(w0, 512 * c)
                        nc.tensor.matmul(
                            ps[:, a:512 * (c + 1)],
                            _r(qkT[qsl, 2 + pair, w0:w0 + 128]),
                            _r(qkT[qsl, pair, a:512 * (c + 1)]),
                            start=True, stop=True)
                for hh in range(2):
                    h = 2 * pair + hh
                    nc.vector.tensor_add(pss[h][:, w0:w0 + 128],
                                         pss[h][:, w0:w0 + 128], masklow)
                for hh in range(2):
                    h = 2 * pair + hh
                    nc.scalar.activation(
                        expTs[h][:, EOFF[j]:EOFF[j] + (S - w0)],
                        pss[h][:, w0:S], AF.Exp, scale=0.125)

        def emit_attv_pass(b, pair, vv, expTs, oT):
            """att@V + denominators for one head pair, j-outer, accumulating
            oT[d, q] and den[q] (replicated across partitions) in PSUM."""
            A = psa.tile([128, S], F32, tag="att", name=f"avA{b}p{pair}")
            Bd = psa.tile([128, S], F32, tag="att", name=f"avB{b}p{pair}")
            h0, h1 = 2 * pair, 2 * pair + 1
            for j in range(ST):
                w0 = 128 * j
                regions = [(w0, 512), (512, S)] if j < 4 else [(w0, S)]
                for (lo, hi) in regions:
                    st = (j == 0)
                    sp_ = (j == 3) if hi <= 512 else (j == ST - 1)
                    # the sim's psum group check is partition-blind; the two
                    # col-tile slots accumulate independent partition ranges
                    for (h, base) in ((h0, 0), (h1, 64)):
                        rsl = slice(EOFF[j] + (lo - w0), EOFF[j] + (hi - w0))
                        nc.tensor.matmul(
                            A[base:base + 64, lo:hi],
                            _r(vv[:, j, 64 * h:64 * h + 64]),
                            _r(expTs[h][:, rsl]), start=st, stop=sp_,
                            skip_group_check=True)
                    for (h, base) in ((h0, 0), (h1, 64)):
                        rsl = slice(EOFF[j] + (lo - w0), EOFF[j] + (hi - w0))
                        nc.tensor.matmul(
                            Bd[base:base + 64, lo:hi],
                            _r(ones_all),
                            _r(expTs[h][:, rsl]), start=st, stop=sp_,
                            skip_group_check=True)
            rec = bp.tile([128, S], F32, tag="rec", name=f"rec{b}p{pair}")
            nc.vector.reciprocal_approx_fast(rec, Bd)
            nc.vector.tensor_mul(oT[:, pair, :], A, rec)

        def emit_out_proj(b, xs, oT, x2, trng):
            for t in trng:
                ps = psa.tile([128, S], F32, tag="att", name=f"op{b}t{t}")
                nc.tensor.matmul(ps[:, 0:E], _r(oT[:, 0, 128 * t:128 * (t + 1)]),
                                 _r(out_w_sb[:, 0, :]), start=True, stop=False)
                nc.tensor.matmul(ps[:, 0:E], _r(oT[:, 1, 128 * t:128 * (t + 1)]),
                                 _r(out_w_sb[:, 1, :]), start=False, stop=False)
                nc.tensor.matmul(ps[:, 0:E], _r(ones_row[0:1, 128 * t:128 * (t + 1)]),
                                 _r(outb_sb[0:1, :]), start=False, stop=True)
                nc.vector.tensor_add(x2[:, t, :], ps[:, 0:E], xs[:, t, :])

        def emit_fc(b, h2T, uT, mrng):
            for m in mrng:
                ps = psa.tile([128, S], F32, tag="att", name=f"fc{b}m{m}")
                for c in range(2):
                    sl = slice(512 * c, 512 * (c + 1))
                    nc.tensor.matmul(ps[:, sl], _r(fc_w_sb[:, 0, 128 * m:128 * (m + 1)]),
                                     _r(h2T[:, 4 * c:4 * c + 4, 0, :]),
                                     start=True, stop=False)
                    nc.tensor.matmul(ps[:, sl], _r(fc_w_sb[:, 1, 128 * m:128 * (m + 1)]),
                                     _r(h2T[:, 4 * c:4 * c + 4, 1, :]),
                                     start=False, stop=True)
                if b == 0:
                    # ACT is busy with element 1's exp stream here: relu on DVE
                    nc.vector.tensor_scalar(
                        out=uT[:, m, :], in0=ps, scalar1=fc_bt_sb[:, m:m + 1],
                        scalar2=0.0, op0=ALU.add, op1=ALU.max)
                else:
                    nc.scalar.activation(uT[:, m, :], ps, AF.Relu,
                                         bias=fc_bt_sb[:, m:m + 1])

        def emit_proj(b, xs, x2, uT, trng, store_halves):
            for t in trng:
                ps = psa.tile([128, S], F32, tag="att", name=f"pj{b}t{t}")
                for m in range(FT):
                    nc.tensor.matmul(ps[:, 0:E], _r(uT[:, m, 128 * t:128 * (t + 1)]),
                                     _r(proj_w_sb[:, m, :]),
                                     start=(m == 0), stop=False)
                nc.tensor.matmul(ps[:, 0:E], _r(ones_row[0:1, 128 * t:128 * (t + 1)]),
                                 _r(projb_sb[0:1, :]), start=False, stop=True)
                nc.vector.tensor_add(xs[:, t, :], ps[:, 0:E], x2[:, t, :])
            for half in store_halves:
                t0 = 4 * half
                nc.sync.dma_start(
                    out=y_out[b, 128 * t0:128 * (t0 + 4), :].rearrange(
                        "(t p) e -> p t e", p=128),
                    in_=xs[:, t0:t0 + 4, :])

        for _rep in range(reps):
            # ---- input DMA (both elements, halves) ----
            xs = []
            for b in range(BPC):
                x_t = bp.tile([128, ST, E], F32, tag="xs", name=f"xs{b}")
                for half in (0, 1):
                    t0 = 4 * half
                    nc.sync.dma_start(
                        out=x_t[:, t0:t0 + 4, :],
                        in_=x_in[b, 128 * t0:128 * (t0 + 4), :].rearrange(
                            "(t p) e -> p t e", p=128))
                xs.append(x_t)

            qkTs = [bp.tile([128, 4, S], MMDT, tag="qkT", name=f"qkT{b}")
                    for b in range(BPC)]
            vvs = [bp.tile([128, ST, E], MMDT, tag="vv", name=f"vv{b}")
                   for b in range(BPC)]
            x2s = [bp.tile([128, ST, E], F32, tag="x2", name=f"x2{b}")
                   for b in range(BPC)]

            # ---- p1(0) ----
            h1T0 = emit_ln(xs[0], "ln1b0")
            emit_late_weights()
            emit_qk(0, h1T0, qkTs[0])
            emit_v(0, h1T0, vvs[0], range(ST))
            # LN1(1): DVE/ACT-rstd/GpSimd/XBAR only -- runs under el0's qk/v
            h1T1 = emit_ln(xs[1], "ln1b1")

            expT0 = {h: ep.tile([128, ETOT], MMDT, tag=f"expT{h}",
                                name=f"expT{h}b0") for h in range(H)}
            # ---- el0 j-loop with el1's p1 matmuls woven in ----
            emit_scores_j(0, 0, qkTs[0], expT0)
            emit_scores_j(0, 1, qkTs[0], expT0)
            emit_qk(1, h1T1, qkTs[1])
            emit_scores_j(0, 2, qkTs[0], expT0)
            emit_v(1, h1T1, vvs[1], range(0, 4))
            emit_scores_j(0, 3, qkTs[0], expT0)
            emit_v(1, h1T1, vvs[1], range(4, ST))
            for j in range(4, ST):
                emit_scores_j(0, j, qkTs[0], expT0)

            # el1 scores chase el0's exp stream on ACT
            expT1 = {h: ep.tile([128, ETOT], MMDT, tag=f"expT{h}",
                                name=f"expT{h}b1") for h in range(H)}
            emit_scores_j(1, 0, qkTs[1], expT1)
            emit_scores_j(1, 1, qkTs[1], expT1)

            # ---- el0 attV + MLP fill the PE during el1's exp window ----
            oT0 = bp.tile([128, ET, S], MMDT, tag="osb", name="oT0")
            emit_attv_pass(0, 0, vvs[0], expT0, oT0)
            emit_attv_pass(0, 1, vvs[0], expT0, oT0)
            emit_scores_j(1, 2, qkTs[1], expT1)
            emit_out_proj(0, xs[0], oT0, x2s[0], range(ST))
            emit_scores_j(1, 3, qkTs[1], expT1)
            h2T0 = emit_ln(x2s[0], "ln2b0")
            uT0 = bp.tile([128, FT, S], MMDT, tag="uT", bufs=1, name="uT0")
            emit_fc(0, h2T0, uT0, range(0, 4))
            emit_scores_j(1, 4, qkTs[1], expT1)
            emit_fc(0, h2T0, uT0, range(4, FT))
            emit_scores_j(1, 5, qkTs[1], expT1)
            emit_proj(0, xs[0], x2s[0], uT0, range(0, 4), [0])
            emit_scores_j(1, 6, qkTs[1], expT1)
            emit_proj(0, xs[0], x2s[0], uT0, range(4, ST), [1])
            emit_scores_j(1, 7, qkTs[1], expT1)

            # ---- el1 attV + MLP tail ----
            oT1 = bp.tile([128, ET, S], MMDT, tag="osb", name="oT1")
            emit_attv_pass(1, 0, vvs[1], expT1, oT1)
            emit_attv_pass(1, 1, vvs[1], expT1, oT1)
            emit_out_proj(1, xs[1], oT1, x2s[1], range(ST))
            h2T1 = emit_ln(x2s[1], "ln2b1")
            uT1 = bp.tile([128, FT, S], MMDT, tag="uT", bufs=1, name="uT1")
            emit_fc(1, h2T1, uT1, range(FT))
            emit_proj(1, xs[1], x2s[1], uT1, range(0, 4), [0])
            emit_proj(1, xs[1], x2s[1], uT1, range(4, ST), [1])

        for p in (psa, psb, ep, bp, sp, wp):
            p.release()

    nc.compile()
    return nc


def host_prep(inputs):
    """Fold LN params into weights; build the DRAM-side weight layouts."""
    f = np.float32
    qkv_w = np.asarray(inputs["qkv_w"], f)
    qkv_b = np.asarray(inputs["qkv_b"], f)
    ln1_g = np.asarray(inputs["ln1_g"], f)
    ln1_b = np.asarray(inputs["ln1_b"], f)
    ln2_g = np.asarray(inputs["ln2_g"], f)
    ln2_b = np.asarray(inputs["ln2_b"], f)
    fc_w = np.asarray(inputs["fc_w"], f)
    fc_b = np.asarray(inputs["fc_b"], f)

    W1 = qkv_w * ln1_g[:, None]
    b1 = qkv_b + ln1_b @ qkv_w
    W2 = fc_w * ln2_g[:, None]
    b2 = fc_b + ln2_b @ fc_w

    qk_w = np.ascontiguousarray(W1[:, :512])
    qk_bc = np.ascontiguousarray(b1[:512].reshape(4, 128).T)
    wv = np.ascontiguousarray(W1[:, 512:768])
    bv = np.ascontiguousarray(b1[512:768].reshape(1, E))
    fc_bt = np.ascontiguousarray(b2.reshape(FT, 128).T)

    import ml_dtypes
    wdt = np.float32 if MM_DTYPE == "f32r" else ml_dtypes.bfloat16

    return {
        "qk_w": qk_w.astype(wdt),
        "qk_bc": qk_bc,
        "wv": wv.astype(wdt),
        "bv_row": bv.astype(wdt),
        "out_w": np.asarray(inputs["out_w"], f).astype(wdt),
        "outb_row": np.asarray(inputs["out_b"], f).reshape(1, E).astype(wdt),
        "fc_w": W2.astype(wdt),
        "fc_bt": fc_bt,
        "proj_w": np.asarray(inputs["proj_w"], f).astype(wdt),
        "projb_row": np.asarray(inputs["proj_b"], f).reshape(1, E).astype(wdt),
    }


_NC_CACHE = None


def _get_nc():
    global _NC_CACHE
    if _NC_CACHE is None:
        _NC_CACHE = build_bass()
    return _NC_CACHE


def run(inputs, trace=False):
    from concourse.bass_utils import run_bass_kernel_spmd

    nc = _get_nc()
    weights = host_prep(inputs)
    x = np.asarray(inputs["x"], np.float32)
    in_maps = []
    for c in range(NCORES):
        m = dict(weights)
        m["x"] = np.ascontiguousarray(x[BPC * c:BPC * (c + 1)])
        in_maps.append(m)
    res = run_bass_kernel_spmd(nc, in_maps, core_ids=list(range(NCORES)),
                               trace=trace)
    y = np.concatenate([res.results[c]["y"] for c in range(NCORES)], axis=0)
    return y, res


def kernel(**inputs):
    y, _ = run(inputs)
    return y


# revision 10
# speedup vs baseline: 1.0576x; 1.0576x over previous
"""Trainium2 Bass kernel for a GPT-style transformer block.

Reference computation (per batch element):
    h  = LN1(x);  qkv = h @ qkv_w + qkv_b
    att = causal_softmax(q @ k.T / sqrt(64));  o = att @ v
    x  = x + o @ out_w + out_b
    h  = LN2(x);  u = relu(h @ fc_w + fc_b)
    y  = x + u @ proj_w + proj_b

Shapes: x [16, 1024, 256], 4 heads x 64, MLP hidden 1024.

Strategy: pure data-parallel over batch, 2 batch elements per core on 8
cores, no collectives.  Within a core:
  - All PE operands are bf16 (fp32 PSUM accumulation).  LN gamma/beta are
    folded into the following matmul weights on the host.
  - Scores are computed transposed (scoresT[k, q]) with the two heads of a
    pair running concurrently as 64-row PE tiles.  The causal mask is a DVE
    add of a strict-lower -1e9 tile onto the diagonal PSUM block before the
    exp (ACT) evacuates psum -> expT (bf16, ragged trapezoid layout).
  - att@V runs j-outer with the v chunks as 64-column stationaries: the two
    heads of a pair occupy the two column-tile slots of the PE and run
    concurrently, accumulating oT[d, q] over j in PSUM.  Softmax
    denominators accumulate in a second PSUM tile from an all-ones [128,64]
    stationary, which replicates den across all 64 partitions so the
    reciprocal and the normalize multiply are single partition-aligned DVE
    ops.  The normalized output lands directly feature-major (oT) for
    out_proj -- no transpose.
  - The two batch elements are software-pipelined: element 1's scores/exp
    chase element 0's exp stream on ACT, while element 0's attV + MLP fill
    the PE during element 1's exp window.  No filler matmuls.
  - PSUM: tag "big" [128,1024]x2 (qk + scores) and tag "att" [128,1024]x2
    (v, attV accumulators, out_proj, fc, proj) = exactly 8 banks.
    Accumulation regions coincide exactly with bank boundaries.
"""

import sys

sys.path.insert(0, "/opt/trn_rl_repo")

import numpy as np

import concourse.bass as bass
import concourse.bacc as bacc
import concourse.tile as tile
from concourse import mybir

# Restrict the activation-table chooser to the one set that contains every
# function this kernel uses (exp, ln, copy/identity, relu) so the ACT
# engine never thrashes table loads.
if not getattr(bacc.get_activation_tables, "_bass_kernel_patched", False):
    _orig_get_act_tables = bacc.get_activation_tables

    def _one_set_tables(module_arch):
        tabs = _orig_get_act_tables(module_arch)
        return {name: (fns if name == "natural_log_exp_and_others" else set())
                for name, fns in tabs.items()}

    _one_set_tables._bass_kernel_patched = True
    bacc.get_activation_tables = _one_set_tables

F32 = mybir.dt.float32
F32R = mybir.dt.float32r
BF16 = mybir.dt.bfloat16
AF = mybir.ActivationFunctionType
ALU = mybir.AluOpType

import os as _os

MM_DTYPE = _os.environ.get("BASS_MM_DTYPE", "bf16")
WARMUP_MM = int(_os.environ.get("BASS_WARMUP_MM", "14"))


def _mmdt():
    return F32R if MM_DTYPE == "f32r" else BF16

NCORES = 8
B = 16
BPC = B // NCORES  # 2 batch elements per core
S = 1024
E = 256
H = 4
D = 64
FF = 1024
ST = S // 128  # 8 seq tiles
ET = E // 128  # 2 feature tiles
FT = FF // 128  # 8 mlp-hidden tiles
EPS = 1e-5

# ragged offsets for the causal expT store: tile j holds q in [128j, S)
EOFF = [0]
for _j in range(ST):
    EOFF.append(EOFF[-1] + (S - 128 * _j))
ETOT = EOFF[ST]  # 4608


def _r(ap):
    """View an fp32 AP as float32r (no-op for bf16/f32r tiles)."""
    if ap.dtype in (F32R, BF16):
        return ap
    return ap.bitcast(F32R)


def build_bass(reps=1):
    MMDT = _mmdt()
    nc = bacc.Bacc(None, target_bir_lowering=False, debug=False)

    # ---- DRAM I/O ----
    x_in = nc.dram_tensor("x", [BPC, S, E], F32, kind="ExternalInput")
    qk_w = nc.dram_tensor("qk_w", [E, 512], MMDT, kind="ExternalInput")
    qk_bc = nc.dram_tensor("qk_bc", [128, 4], F32, kind="ExternalInput")
    wv = nc.dram_tensor("wv", [E, E], MMDT, kind="ExternalInput")
    bv_row = nc.dram_tensor("bv_row", [1, E], MMDT, kind="ExternalInput")
    out_w = nc.dram_tensor("out_w", [E, E], MMDT, kind="ExternalInput")
    outb_row = nc.dram_tensor("outb_row", [1, E], MMDT, kind="ExternalInput")
    fc_w = nc.dram_tensor("fc_w", [E, FF], MMDT, kind="ExternalInput")
    fc_bt = nc.dram_tensor("fc_bt", [128, FT], F32, kind="ExternalInput")
    proj_w = nc.dram_tensor("proj_w", [FF, E], MMDT, kind="ExternalInput")
    projb_row = nc.dram_tensor("projb_row", [1, E], MMDT, kind="ExternalInput")
    y_out = nc.dram_tensor("y", [BPC, S, E], F32, kind="ExternalOutput")

    with tile.TileContext(nc) as tc:
        wp = tc.alloc_tile_pool(name="weights", bufs=1)
        sp = tc.alloc_tile_pool(name="small", bufs=2)
        bp = tc.alloc_tile_pool(name="big", bufs=2)
        ep = tc.alloc_tile_pool(name="expt", bufs=2)
        psb = tc.alloc_tile_pool(name="psbig", bufs=2, space="PSUM")
        psa = tc.alloc_tile_pool(name="psatt", bufs=2, space="PSUM")

        # ---- constants first: gpsimd builds the mask before anything else ----
        eps_col = wp.tile([128, 1], F32)
        nc.vector.memset(eps_col, EPS)
        ones_row = wp.tile([1, S], MMDT)
        nc.vector.memset(ones_row, 1.0)
        ones_all = wp.tile([128, D], MMDT)
        nc.vector.memset(ones_all, 1.0)
        # multiplicative causal mask for the expT diagonal block: 0 where
        # q < k (col < partition), 1 elsewhere; applied on GpSimd in SBUF
        mask01 = wp.tile([128, 128], MMDT)
        nc.gpsimd.memset(mask01, 1.0)
        nc.gpsimd.affine_select(
            out=mask01, in_=mask01, compare_op=ALU.is_ge, fill=0.0,
            base=0, channel_multiplier=-1, pattern=[[1, 128]])

        # ---- PE warmup: keep the HAM clock gate open during input DMA ----
        warm = wp.tile([128, 512], MMDT)
        nc.vector.memset(warm, 0.25)
        if WARMUP_MM:
            for _ in range(WARMUP_MM):
                wps = psa.tile([128, S], F32, tag="att")
                nc.tensor.matmul(wps[:, 0:512], _r(warm[:, 0:128]), _r(warm),
                                 start=True, stop=True)

        # ---- weights needed early (attention input projections) ----
        qk_w_sb = wp.tile([128, ET, 512], MMDT)
        nc.gpsimd.dma_start(out=qk_w_sb, in_=qk_w[:, :].rearrange("(t p) c -> p t c", p=128))
        qk_bc_sb = wp.tile([128, 4], F32)
        nc.gpsimd.dma_start(out=qk_bc_sb, in_=qk_bc[:, :])
        wv_sb = wp.tile([128, ET, E], MMDT)
        nc.gpsimd.dma_start(out=wv_sb, in_=wv[:, :].rearrange("(t p) c -> p t c", p=128))
        bv_sb = wp.tile([1, E], MMDT)
        nc.gpsimd.dma_start(out=bv_sb, in_=bv_row[:, :])

        # late weights: tiles allocated now, DMAs emitted after p1(0)
        out_w_sb = wp.tile([128, ET, E], MMDT)
        outb_sb = wp.tile([1, E], MMDT)
        fc_w_sb = wp.tile([128, ET, FF], MMDT)
        fc_bt_sb = wp.tile([128, FT], F32)
        proj_w_sb = wp.tile([128, FT, E], MMDT)
        projb_sb = wp.tile([1, E], MMDT)

        def emit_late_weights():
            nc.gpsimd.dma_start(out=out_w_sb, in_=out_w[:, :].rearrange("(t p) c -> p t c", p=128))
            nc.gpsimd.dma_start(out=outb_sb, in_=outb_row[:, :])
            nc.gpsimd.dma_start(out=fc_w_sb, in_=fc_w[:, :].rearrange("(t p) c -> p t c", p=128))
            nc.gpsimd.dma_start(out=fc_bt_sb, in_=fc_bt[:, :])
            nc.gpsimd.dma_start(out=proj_w_sb, in_=proj_w[:, :].rearrange("(t p) c -> p t c", p=128))
            nc.gpsimd.dma_start(out=projb_sb, in_=projb_row[:, :])

        def emit_ln_half(src, dst_fn, half, rstd, nmr):
            """LayerNorm of seq tiles [4*half, 4*half+4): stats on DVE, rstd
            on ACT, apply on GpSimd -> bf16 dst."""
            t0 = 4 * half
            stats = sp.tile([128, 4, 6], F32, tag="bnstats")
            mv = sp.tile([128, 4, 2], F32, tag="bnaggr")
            for i in range(4):
                nc.vector.bn_stats(out=stats[:, i, :], in_=src[:, t0 + i, :])
                nc.vector.bn_aggr(out=mv[:, i, :], in_=stats[:, i, :])
            sl = slice(t0, t0 + 4)
            # rstd = exp(-0.5 * ln(var + eps))
            nc.scalar.activation(rstd[:, sl], mv[:, :, 1], AF.Ln, bias=eps_col)
            nc.scalar.activation(rstd[:, sl], rstd[:, sl], AF.Exp, scale=-0.5)
            nc.vector.tensor_mul(nmr[:, sl], mv[:, :, 0], rstd[:, sl])
            nc.vector.tensor_scalar_mul(nmr[:, sl], nmr[:, sl], -1.0)
            for i in range(4):
                t = t0 + i
                nc.gpsimd.tensor_scalar(
                    out=dst_fn(t), in0=src[:, t, :],
                    scalar1=rstd[:, t:t + 1], scalar2=nmr[:, t:t + 1],
                    op0=ALU.mult, op1=ALU.add)

        def emit_transpose_half(h_sb, hT, half):
            """XBAR transpose seq tiles [4h, 4h+4) of h_sb [128, ST, E] into
            hT [128, ST, ET, 128] (layout [e_lo, t, et, q])."""
            t0 = 4 * half
            nc.sync.dma_start_transpose(
                out=hT[:, t0:t0 + 4, :, :], in_=h_sb[:, t0:t0 + 4, :])

        def emit_ln(src, name):
            """Full LN -> (h bf16, hT transposed) using ring tags h/hT."""
            h = bp.tile([128, ST, E], MMDT, tag="h", name=f"h{name}")
            hT = bp.tile([128, ST, ET, 128], MMDT, tag="hT", name=f"hT{name}")
            rstd = sp.tile([128, ST], F32, tag="rstd")
            nmr = sp.tile([128, ST], F32, tag="nmr")
            for half in (0, 1):
                emit_ln_half(src, lambda t: h[:, t, :], half, rstd, nmr)
                emit_transpose_half(h, hT, half)
            return hT

        def emit_qk(b, hT, qkT):
            # q/k feature-major: qkT[:, m, :]; m=0: q heads 0-1, m=1: q heads
            # 2-3, m=2: k heads 0-1, m=3: k heads 2-3 (rows = 2x64 head dims)
            for m in range(4):
                ps = psb.tile([128, S], F32, tag="big", name=f"qk{b}m{m}")
                for c in range(2):
                    sl = slice(512 * c, 512 * (c + 1))
                    nc.tensor.matmul(ps[:, sl], _r(qk_w_sb[:, 0, 128 * m:128 * (m + 1)]),
                                     _r(hT[:, 4 * c:4 * c + 4, 0, :]),
                                     start=True, stop=False)
                    nc.tensor.matmul(ps[:, sl], _r(qk_w_sb[:, 1, 128 * m:128 * (m + 1)]),
                                     _r(hT[:, 4 * c:4 * c + 4, 1, :]),
                                     start=False, stop=True)
                    nc.vector.tensor_scalar(
                        out=qkT[:, m, sl], in0=ps[:, sl],
                        scalar1=qk_bc_sb[:, m:m + 1],
                        scalar2=None, op0=ALU.add)

        def emit_v(b, hT, vv, trng):
            # v seq-major: vv[:, t, 64h:64h+64] per head
            for t in trng:
                ps = psa.tile([128, S], F32, tag="att", name=f"v{b}t{t}")
                nc.tensor.matmul(ps[:, 0:E], _r(hT[:, t, 0, :]),
                                 _r(wv_sb[:, 0, :]), start=True, stop=False)
                nc.tensor.matmul(ps[:, 0:E], _r(hT[:, t, 1, :]),
                                 _r(wv_sb[:, 1, :]), start=False, stop=False)
                nc.tensor.matmul(ps[:, 0:E], _r(ones_row[0:1, 128 * t:128 * (t + 1)]),
                                 _r(bv_sb[0:1, :]), start=False, stop=True)
                nc.vector.tensor_copy(vv[:, t, :], ps[:, 0:E])

        def emit_scores_pair(b, j, pair, qkT, expTs):
            """Scores + exp + gpsimd mask for k-tile j, one head pair."""
            w0 = 128 * j
            pss = {}
            for hh in range(2):
                h = 2 * pair + hh
                qsl = slice(64 * hh, 64 * hh + 64)
                ps = psb.tile([128, S], F32, tag="big", name=f"sc{b}h{h}j{j}")
                pss[h] = ps
                for c in range(w0 // 512, 2):
                    a = max(w0, 512 * c)
                    nc.tensor.matmul(
                        ps[:, a:512 * (c + 1)],
                        _r(qkT[qsl, 2 + pair, w0:w0 + 128]),
                        _r(qkT[qsl, pair, a:512 * (c + 1)]),
                        start=True, stop=True)
            for hh in range(2):
                h = 2 * pair + hh
                nc.scalar.activation(
                    expTs[h][:, EOFF[j]:EOFF[j] + (S - w0)],
                    pss[h][:, w0:S], AF.Exp, scale=0.125)
            for hh in range(2):
                h = 2 * pair + hh
                # zero the invalid (q < k) triangle of the diagonal block
                nc.gpsimd.tensor_mul(expTs[h][:, EOFF[j]:EOFF[j] + 128],
                                     expTs[h][:, EOFF[j]:EOFF[j] + 128],
                                     mask01)

        def attv_open(b, pair):
            A = psa.tile([128, S], F32, tag="att", name=f"avA{b}p{pair}")
            Bd = psa.tile([128, S], F32, tag="att", name=f"avB{b}p{pair}")
            return A, Bd

        def attv_chunk(b, pair, AB, vv, expTs, jrng):
            """att@V + denominators for one head pair, k-tiles jrng,
            accumulating oT[d, q] and den[q] (replicated across the 64
            output partitions) in PSUM."""
            A, Bd = AB
            h0, h1 = 2 * pair, 2 * pair + 1
            for j in jrng:
                w0 = 128 * j
                regions = [(w0, 512), (512, S)] if j < 4 else [(w0, S)]
                for (lo, hi) in regions:
                    st = (j == 0)
                    sp_ = (j == 3) if hi <= 512 else (j == ST - 1)
                    # the sim's psum group check is partition-blind; the two
                    # col-tile slots accumulate independent partition ranges
                    for (h, base) in ((h0, 0), (h1, 64)):
                        rsl = slice(EOFF[j] + (lo - w0), EOFF[j] + (hi - w0))
                        nc.tensor.matmul(
                            A[base:base + 64, lo:hi],
                            _r(vv[:, j, 64 * h:64 * h + 64]),
                            _r(expTs[h][:, rsl]), start=st, stop=sp_,
                            skip_group_check=True)
                    for (h, base) in ((h0, 0), (h1, 64)):
                        rsl = slice(EOFF[j] + (lo - w0), EOFF[j] + (hi - w0))
                        nc.tensor.matmul(
                            Bd[base:base + 64, lo:hi],
                            _r(ones_all),
                            _r(expTs[h][:, rsl]), start=st, stop=sp_,
                            skip_group_check=True)

        def attv_close(b, pair, AB, oT):
            A, Bd = AB
            rec = bp.tile([128, S], F32, tag="rec", name=f"rec{b}p{pair}")
            nc.vector.reciprocal_approx_fast(rec, Bd)
            nc.vector.tensor_mul(oT[:, pair, :], A, rec)

        def emit_fil(n):
            """HAM-warming filler matmuls (att psum ring, no consumers)."""
            fps = psa.tile([128, S], F32, tag="att", name="fil")
            for _ in range(n):
                nc.tensor.matmul(fps[:, 0:256], _r(warm[:, 0:128]),
                                 _r(warm[:, 0:256]), start=True, stop=True)

        def emit_out_proj(b, xs, oT, x2, trng):
            for t in trng:
                ps = psa.tile([128, S], F32, tag="att", name=f"op{b}t{t}")
                nc.tensor.matmul(ps[:, 0:E], _r(oT[:, 0, 128 * t:128 * (t + 1)]),
                                 _r(out_w_sb[:, 0, :]), start=True, stop=False)
                nc.tensor.matmul(ps[:, 0:E], _r(oT[:, 1, 128 * t:128 * (t + 1)]),
                                 _r(out_w_sb[:, 1, :]), start=False, stop=False)
                nc.tensor.matmul(ps[:, 0:E], _r(ones_row[0:1, 128 * t:128 * (t + 1)]),
                                 _r(outb_sb[0:1, :]), start=False, stop=True)
                nc.vector.tensor_add(x2[:, t, :], ps[:, 0:E], xs[:, t, :])

        def emit_fc(b, h2T, uT, mrng):
            for m in mrng:
                ps = psa.tile([128, S], F32, tag="att", name=f"fc{b}m{m}")
                for c in range(2):
                    sl = slice(512 * c, 512 * (c + 1))
                    nc.tensor.matmul(ps[:, sl], _r(fc_w_sb[:, 0, 128 * m:128 * (m + 1)]),
                                     _r(h2T[:, 4 * c:4 * c + 4, 0, :]),
                                     start=True, stop=False)
                    nc.tensor.matmul(ps[:, sl], _r(fc_w_sb[:, 1, 128 * m:128 * (m + 1)]),
                                     _r(h2T[:, 4 * c:4 * c + 4, 1, :]),
                                     start=False, stop=True)
                if b == 0:
                    # ACT is busy with element 1's exp stream here: relu on DVE
                    nc.vector.tensor_scalar(
                        out=uT[:, m, :], in0=ps, scalar1=fc_bt_sb[:, m:m + 1],
                        scalar2=0.0, op0=ALU.add, op1=ALU.max)
                else:
                    nc.scalar.activation(uT[:, m, :], ps, AF.Relu,
                                         bias=fc_bt_sb[:, m:m + 1])

        def emit_proj(b, xs, x2, uT, trng, store_halves):
            for t in trng:
                ps = psa.tile([128, S], F32, tag="att", name=f"pj{b}t{t}")
                for m in range(FT):
                    nc.tensor.matmul(ps[:, 0:E], _r(uT[:, m, 128 * t:128 * (t + 1)]),
                                     _r(proj_w_sb[:, m, :]),
                                     start=(m == 0), stop=False)
                nc.tensor.matmul(ps[:, 0:E], _r(ones_row[0:1, 128 * t:128 * (t + 1)]),
                                 _r(projb_sb[0:1, :]), start=False, stop=True)
                nc.vector.tensor_add(xs[:, t, :], ps[:, 0:E], x2[:, t, :])
            for half in store_halves:
                t0 = 4 * half
                nc.sync.dma_start(
                    out=y_out[b, 128 * t0:128 * (t0 + 4), :].rearrange(
                        "(t p) e -> p t e", p=128),
                    in_=xs[:, t0:t0 + 4, :])

        for _rep in range(reps):
            # ---- input DMA (both elements, halves) ----
            xs = []
            for b in range(BPC):
                x_t = bp.tile([128, ST, E], F32, tag="xs", name=f"xs{b}")
                for half in (0, 1):
                    t0 = 4 * half
                    nc.sync.dma_start(
                        out=x_t[:, t0:t0 + 4, :],
                        in_=x_in[b, 128 * t0:128 * (t0 + 4), :].rearrange(
                            "(t p) e -> p t e", p=128))
                xs.append(x_t)

            qkTs = [bp.tile([128, 4, S], MMDT, tag="qkT", name=f"qkT{b}")
                    for b in range(BPC)]
            vvs = [bp.tile([128, ST, E], MMDT, tag="vv", name=f"vv{b}")
                   for b in range(BPC)]
            x2s = [bp.tile([128, ST, E], F32, tag="x2", name=f"x2{b}")
                   for b in range(BPC)]

            # ---- p1(0) ----
            h1T0 = emit_ln(xs[0], "ln1b0")
            emit_late_weights()
            emit_qk(0, h1T0, qkTs[0])
            emit_v(0, h1T0, vvs[0], range(ST))
            # LN1(1): DVE/ACT-rstd/GpSimd/XBAR only -- runs under el0's qk/v
            h1T1 = emit_ln(xs[1], "ln1b1")

            expT0 = {h: ep.tile([128, ETOT], MMDT, tag=f"expT{h}",
                                name=f"expT{h}b0") for h in range(H)}
            # ---- el0 j-loop: exp(0) paces ACT; el1's qk/v + fillers keep the
            # PE dense through the exp waits (HAM must stay warm) ----
            def emit_qk1_m(m):
                ps = psb.tile([128, S], F32, tag="big", name=f"qk1m{m}")
                for c in range(2):
                    sl = slice(512 * c, 512 * (c + 1))
                    nc.tensor.matmul(ps[:, sl], _r(qk_w_sb[:, 0, 128 * m:128 * (m + 1)]),
                                     _r(h1T1[:, 4 * c:4 * c + 4, 0, :]),
                                     start=True, stop=False)
                    nc.tensor.matmul(ps[:, sl], _r(qk_w_sb[:, 1, 128 * m:128 * (m + 1)]),
                                     _r(h1T1[:, 4 * c:4 * c + 4, 1, :]),
                                     start=False, stop=True)
                    nc.vector.tensor_scalar(
                        out=qkTs[1][:, m, sl], in0=ps[:, sl],
                        scalar1=qk_bc_sb[:, m:m + 1], scalar2=None, op0=ALU.add)

            # slot plan: (j,pair) -> list of closures
            plan0 = {
                (0, 0): [lambda: emit_qk1_m(0), lambda: emit_fil(3)],
                (0, 1): [lambda: emit_qk1_m(1), lambda: emit_fil(3)],
                (1, 0): [lambda: emit_qk1_m(2), lambda: emit_fil(2)],
                (1, 1): [lambda: emit_qk1_m(3), lambda: emit_fil(2)],
                (2, 0): [lambda: emit_v(1, h1T1, vvs[1], range(0, 2)),
                         lambda: emit_fil(2)],
                (2, 1): [lambda: emit_v(1, h1T1, vvs[1], range(2, 4)),
                         lambda: emit_fil(2)],
                (3, 0): [lambda: emit_v(1, h1T1, vvs[1], range(4, 6)),
                         lambda: emit_fil(1)],
                (3, 1): [lambda: emit_v(1, h1T1, vvs[1], range(6, 8)),
                         lambda: emit_fil(1)],
                (4, 0): [lambda: emit_fil(3)],
                (4, 1): [lambda: emit_fil(3)],
                (5, 0): [lambda: emit_fil(2)],
                (5, 1): [lambda: emit_fil(2)],
                (6, 0): [lambda: emit_fil(2)],
                (6, 1): [lambda: emit_fil(1)],
                (7, 0): [lambda: emit_fil(1)],
                (7, 1): [],
            }
            for j in range(ST):
                for pair in range(2):
                    emit_scores_pair(0, j, pair, qkTs[0], expT0)
                    for w in plan0[(j, pair)]:
                        w()

            # ---- el1 j-loop: exp(1) paces ACT; el0's attV chunks + MLP
            # fill the PE ----
            expT1 = {h: ep.tile([128, ETOT], MMDT, tag=f"expT{h}",
                                name=f"expT{h}b1") for h in range(H)}
            oT0 = bp.tile([128, ET, S], MMDT, tag="osb", name="oT0")
            uT0 = bp.tile([128, FT, S], MMDT, tag="uT", bufs=1, name="uT0")
            st1 = {"AB": None, "h2T0": None}

            def av0(pair, jrng, close=False):
                if st1["AB"] is None:
                    st1["AB"] = attv_open(0, pair)
                attv_chunk(0, pair, st1["AB"], vvs[0], expT0, jrng)
                if close:
                    attv_close(0, pair, st1["AB"], oT0)
                    st1["AB"] = None

            def ln2_0():
                st1["h2T0"] = emit_ln(x2s[0], "ln2b0")

            plan1 = {
                (0, 0): [lambda: av0(0, range(0, 2))],
                (0, 1): [lambda: av0(0, range(2, 4))],
                (1, 0): [lambda: av0(0, range(4, 6))],
                (1, 1): [lambda: av0(0, range(6, 8), close=True)],
                (2, 0): [lambda: av0(1, range(0, 2))],
                (2, 1): [lambda: av0(1, range(2, 4))],
                (3, 0): [lambda: av0(1, range(4, 6))],
                (3, 1): [lambda: av0(1, range(6, 8), close=True)],
                (4, 0): [lambda: emit_out_proj(0, xs[0], oT0, x2s[0], range(0, 4))],
                (4, 1): [lambda: emit_out_proj(0, xs[0], oT0, x2s[0], range(4, 8)),
                         ln2_0],
                (5, 0): [lambda: emit_fil(2)],
                (5, 1): [lambda: emit_fc(0, st1["h2T0"], uT0, range(0, 2))],
                (6, 0): [lambda: emit_fc(0, st1["h2T0"], uT0, range(2, 4))],
                (6, 1): [lambda: emit_fc(0, st1["h2T0"], uT0, range(4, 6))],
                (7, 0): [lambda: emit_fc(0, st1["h2T0"], uT0, range(6, 8))],
                (7, 1): [lambda: emit_proj(0, xs[0], x2s[0], uT0, range(0, 2), [])],
            }
            for j in range(ST):
                for pair in range(2):
                    emit_scores_pair(1, j, pair, qkTs[1], expT1)
                    for w in plan1[(j, pair)]:
                        w()

            # ---- tail: el1 attV + both elements' remaining MLP ----
            oT1 = bp.tile([128, ET, S], MMDT, tag="osb", name="oT1")
            AB = attv_open(1, 0)
            attv_chunk(1, 0, AB, vvs[1], expT1, range(ST))
            attv_close(1, 0, AB, oT1)
            AB = attv_open(1, 1)
            attv_chunk(1, 1, AB, vvs[1], expT1, range(ST))
            attv_close(1, 1, AB, oT1)
            emit_proj(0, xs[0], x2s[0], uT0, range(2, 4), [0])
            emit_out_proj(1, xs[1], oT1, x2s[1], range(ST))
            h2T1 = emit_ln(x2s[1], "ln2b1")
            emit_proj(0, xs[0], x2s[0], uT0, range(4, ST), [1])
            uT1 = bp.tile([128, FT, S], MMDT, tag="uT", bufs=1, name="uT1")
            emit_fc(1, h2T1, uT1, range(FT))
            emit_proj(1, xs[1], x2s[1], uT1, range(0, 4), [0])
            emit_proj(1, xs[1], x2s[1], uT1, range(4, ST), [1])

        for p in (psa, psb, ep, bp, sp, wp):
            p.release()

    nc.compile()
    return nc


def host_prep(inputs):
    """Fold LN params into weights; build the DRAM-side weight layouts."""
    f = np.float32
    qkv_w = np.asarray(inputs["qkv_w"], f)
    qkv_b = np.asarray(inputs["qkv_b"], f)
    ln1_g = np.asarray(inputs["ln1_g"], f)
    ln1_b = np.asarray(inputs["ln1_b"], f)
    ln2_g = np.asarray(inputs["ln2_g"], f)
    ln2_b = np.asarray(inputs["ln2_b"], f)
    fc_w = np.asarray(inputs["fc_w"], f)
    fc_b = np.asarray(inputs["fc_b"], f)

    W1 = qkv_w * ln1_g[:, None]
    b1 = qkv_b + ln1_b @ qkv_w
    W2 = fc_w * ln2_g[:, None]
    b2 = fc_b + ln2_b @ fc_w

    qk_w = np.ascontiguousarray(W1[:, :512])
    qk_bc = np.ascontiguousarray(b1[:512].reshape(4, 128).T)
    wv = np.ascontiguousarray(W1[:, 512:768])
    bv = np.ascontiguousarray(b1[512:768].reshape(1, E))
    fc_bt = np.ascontiguousarray(b2.reshape(FT, 128).T)

    import ml_dtypes
    wdt = np.float32 if MM_DTYPE == "f32r" else ml_dtypes.bfloat16

    return {
        "qk_w": qk_w.astype(wdt),
        "qk_bc": qk_bc,
        "wv": wv.astype(wdt),
        "bv_row": bv.astype(wdt),
        "out_w": np.asarray(inputs["out_w"], f).astype(wdt),
        "outb_row": np.asarray(inputs["out_b"], f).reshape(1, E).astype(wdt),
        "fc_w": W2.astype(wdt),
        "fc_bt": fc_bt,
        "proj_w": np.asarray(inputs["proj_w"], f).astype(wdt),
        "projb_row": np.asarray(inputs["proj_b"], f).reshape(1, E).astype(wdt),
    }


_NC_CACHE = None


def _get_nc():
    global _NC_CACHE
    if _NC_CACHE is None:
        _NC_CACHE = build_bass()
    return _NC_CACHE


def run(inputs, trace=False):
    from concourse.bass_utils import run_bass_kernel_spmd

    nc = _get_nc()
    weights = host_prep(inputs)
    x = np.asarray(inputs["x"], np.float32)
    in_maps = []
    for c in range(NCORES):
        m = dict(weights)
        m["x"] = np.ascontiguousarray(x[BPC * c:BPC * (c + 1)])
        in_maps.append(m)
    res = run_bass_kernel_spmd(nc, in_maps, core_ids=list(range(NCORES)),
                               trace=trace)
    y = np.concatenate([res.results[c]["y"] for c in range(NCORES)], axis=0)
    return y, res


def kernel(**inputs):
    y, _ = run(inputs)
    return y
